# revision 3
# baseline (speedup 1.0000x reference)
"""GCNConv (PyG semantics: normalize=True, add_self_loops=True, edge_weight)
as a Trainium2 Bass kernel, SPMD over 8 NeuronCores.

Strategy: shard destination nodes across the 8 cores (per sharding hint).
The normalized adjacency (with self loops), A[dst, src] = dinv[src]*w*dinv[dst],
is materialized on the host as a dense bf16 matrix, pre-transposed and
pre-swizzled into the exact SBUF tile layout each core consumes. On device,
each core computes  agg = A_shard @ x  as a dense TensorEngine sweep:
x lives SBUF-resident in bf16 ([128, 80, 512], k-major), and per 128-dst
block the PE accumulates 80 k-tile matmuls into PSUM while the next block's
A^T stripe streams from DRAM. The transform  out = agg @ W + b  then runs in
fp32 per block: PE transposes agg (4x 128x128), multiplies by W with
contraction over in-features, and DVE adds the bias on the PSUM->SBUF copy.

The adjacency build (degree normalization folded into matrix values) is
host-side index preprocessing; every O(N*N*D) / O(N*D^2) FLOP runs on device.
A dense sweep is deliberate: the fast SWDGE gather ucode is unavailable on
this runtime and per-row indirect DMA measures ~17 GB/s, while the PE sweep
sustains the full array throughput.
"""
from contextlib import ExitStack

import numpy as np
import ml_dtypes

import concourse.bacc as bacc
import concourse.mybir as mybir
import concourse.tile as tile
from concourse.bass_utils import run_bass_kernel_spmd

P = 128
CORES = 8
BF16 = mybir.dt.bfloat16
F32 = mybir.dt.float32


def _preprocess(x, edge_index, edge_attr):
    """Self loops, symmetric normalization, and the per-core dense A^T
    stripes in SBUF-swizzled layout: at_sw[c][g, p, k*P+m] = A^T[k*P+p (src),
    c*BPC*P + g*P + m (dst)]."""
    n = x.shape[0]
    src = np.asarray(edge_index[0], np.int64)
    dst = np.asarray(edge_index[1], np.int64)
    loop = np.arange(n, dtype=np.int64)
    src_f = np.concatenate([src, loop])
    dst_f = np.concatenate([dst, loop])
    ew = np.concatenate(
        [np.asarray(edge_attr, np.float64), np.ones(n, np.float64)])

    deg = np.zeros(n, np.float64)
    np.add.at(deg, dst_f, ew)
    dinv = np.where(deg > 0, 1.0 / np.sqrt(np.maximum(deg, 1e-300)), 0.0)
    sc = (dinv[src_f] * ew * dinv[dst_f]).astype(np.float32)

    bpc = -(-n // (CORES * P))           # dst blocks per core
    npad = CORES * bpc * P               # padded node count (dst slots)
    kt = -(-npad // P)                   # k-tiles over (padded) src nodes
    assert kt * P == npad

    core_of = dst_f // (bpc * P)
    g_of = (dst_f % (bpc * P)) // P
    m_of = dst_f % P
    p_of = src_f % P
    col_of = (src_f // P) * P + m_of

    at_sw = np.zeros((CORES, bpc, P, kt * P), np.float32)
    np.add.at(at_sw, (core_of, g_of, p_of, col_of), sc)
    at_sw = at_sw.astype(ml_dtypes.bfloat16)

    return dict(bpc=bpc, npad=npad, kt=kt, at_sw=at_sw)


def _build_module(n, d_in, d_out, bpc, kt, reps=1):
    """Emit the SPMD per-core Bass program."""
    assert d_in % P == 0 and d_out % P == 0
    kt_w = d_in // P
    npad = kt * P

    nc = bacc.Bacc("TRN2", target_bir_lowering=False, debug=False)
    x_d = nc.dram_tensor("x", [npad, d_in], BF16, kind="ExternalInput")
    at_d = nc.dram_tensor("at", [bpc, P, kt * P], BF16, kind="ExternalInput")
    W_d = nc.dram_tensor("W", [d_in, d_out], F32, kind="ExternalInput")
    bias_d = nc.dram_tensor("bias", [P, d_out], F32, kind="ExternalInput")
    ident_d = nc.dram_tensor("ident", [P, P], F32, kind="ExternalInput")
    out_d = nc.dram_tensor("out", [bpc, P, d_out], F32, kind="ExternalOutput")

    with tile.TileContext(nc) as tc, ExitStack() as ctx:
        const = ctx.enter_context(tc.tile_pool(name="const", bufs=1))
        atp = ctx.enter_context(tc.tile_pool(name="atp", bufs=3))
        apool = ctx.enter_context(tc.tile_pool(name="aggsb", bufs=2))
        tpool = ctx.enter_context(tc.tile_pool(name="atsb", bufs=8))
        opool = ctx.enter_context(tc.tile_pool(name="outsb", bufs=2))
        ps_agg = ctx.enter_context(tc.tile_pool(name="ps_agg", bufs=2, space="PSUM"))
        ps_t = ctx.enter_context(tc.tile_pool(name="ps_t", bufs=4, space="PSUM"))
        ps_out = ctx.enter_context(tc.tile_pool(name="ps_out", bufs=2, space="PSUM"))

        x_sb = const.tile([P, kt, d_in], BF16)
        nc.sync.dma_start(x_sb[:], x_d.ap().rearrange("(k p) d -> p k d", p=P))
        W_sb = const.tile([P, kt_w, d_out], F32)
        nc.sync.dma_start(W_sb[:], W_d.ap().rearrange("(k p) d -> p k d", p=P))
        bias_sb = const.tile([P, d_out], F32)
        nc.sync.dma_start(bias_sb[:], bias_d[:, :])
        ident_sb = const.tile([P, P], F32)
        nc.sync.dma_start(ident_sb[:], ident_d[:, :])

        for _ in range(reps):
            for g in range(bpc):
                at_sb = atp.tile([P, kt, P], BF16, tag="at")
                nc.sync.dma_start(at_sb[:], at_d[g].rearrange("p (k m) -> p k m", m=P))
                agg_ps = ps_agg.tile([P, d_in], F32)
                for k in range(kt):
                    nc.tensor.matmul(agg_ps[:], at_sb[:, k, :], x_sb[:, k, :],
                                     start=(k == 0), stop=(k == kt - 1))
                agg_sb = apool.tile([P, d_in], F32, tag="agg")
                nc.scalar.copy(agg_sb[:], agg_ps[:])
                out_ps = ps_out.tile([P, d_out], F32)
                for ki in range(kt_w):
                    pt = ps_t.tile([P, P], F32, tag="pt")
                    nc.tensor.transpose(pt[:], agg_sb[:, ki * P:(ki + 1) * P],
                                        ident_sb[:])
                    aT = tpool.tile([P, P], F32, tag="aT")
                    nc.scalar.copy(aT[:], pt[:])
                    nc.tensor.matmul(out_ps[:], aT[:], W_sb[:, ki, :],
                                     start=(ki == 0), stop=(ki == kt_w - 1))
                out_sb = opool.tile([P, d_out], F32, tag="out")
                nc.vector.tensor_add(out_sb[:], out_ps[:], bias_sb[:])
                nc.sync.dma_start(out_d[g], out_sb[:])

    nc.compile()
    return nc


def _make_in_maps(x, W, b, pre):
    n, d_in = np.asarray(x).shape
    npad = pre["npad"]
    x16 = np.zeros((npad, d_in), ml_dtypes.bfloat16)
    x16[:n] = np.asarray(x, np.float32).astype(ml_dtypes.bfloat16)
    W32 = np.ascontiguousarray(np.asarray(W, np.float32))
    bias_bcast = np.ascontiguousarray(
        np.tile(np.asarray(b, np.float32)[None, :], (P, 1)))
    ident32 = np.eye(P, dtype=np.float32)
    return [
        dict(x=x16, at=np.ascontiguousarray(pre["at_sw"][c]),
             W=W32, bias=bias_bcast, ident=ident32)
        for c in range(CORES)
    ]


def kernel(x, edge_index, edge_attr, W, b):
    x = np.asarray(x)
    n, d_in = x.shape
    d_out = np.asarray(W).shape[1]
    pre = _preprocess(x, edge_index, edge_attr)
    nc = _build_module(n, d_in, d_out, pre["bpc"], pre["kt"])
    in_maps = _make_in_maps(x, W, b, pre)
    res = run_bass_kernel_spmd(nc, in_maps, list(range(CORES)))
    out_all = np.concatenate([res.results[c]["out"] for c in range(CORES)],
                             axis=0)            # [CORES*bpc, P, d_out]
    out = out_all.reshape(-1, d_out)[:n]
    return np.ascontiguousarray(out.astype(np.float32))
